# revision 16
# baseline (speedup 1.0000x reference)
"""Exact KNN collision kernel for trn2 (8 NeuronCores).

nn[b,n] = argmin_m |vertices[b,n] - collider[b, cvi[m]]|^2, bit-exact fp32
scores with first-occurrence tie-breaking (matches the jax reference).

Per core (core c -> batch b=c//2, row-half h=c%2, 8192 rows, 64 tiles of 128):
  - PE: s = [v;1]^T @ [c; -|c|^2/2]  (K=4 fp32r matmul -> PSUM chunks)
    argmax_m s == argmin_m d2 exactly (d2 = -2s in fp32).
  - pass 1: running-max scan of s along candidates (tensor_tensor_scan max,
    split DVE/Pool, phase-chained via initial= carry) -> sc in SBUF.
  - r = max(r_dve_region, r_pool_region).
  - pass 2: counts of (sc < r) per chunk: ACT (Sign activation, scale=-1,
    bias=r, sum-accumulator) + DVE/Pool (scalar_tensor_tensor is_lt).
    Since sc is monotone per region, count == position of first occurrence;
    chunk counts compose: k = cnt1 + [cnt1==len1]*(cnt2 + [cnt2==len2]*(...)).
  - host maps dedup slot -> first position in collision_vertices.
"""
import os
import sys
import numpy as np

_BASS_PATH = "/opt/trn_rl_repo"
if _BASS_PATH not in sys.path:
    sys.path.insert(0, _BASS_PATH)

B, N, V, M = 4, 16384, 6890, 4096
NCORES = 8
ROWS = (B * N) // NCORES          # 8192 rows per core
NT = ROWS // 128                  # 64 row tiles

MM_DTYPE = os.environ.get("KNN_MM_DTYPE", "float32r")

_PROGRAM_CACHE = {}


def _splits(U):
    """W = padded total columns (even halves); HALF per scan stream."""
    W = ((U + 255) // 256) * 256
    return W, W // 2


STRIDE = int(os.environ.get("KNN_STRIDE", "16"))   # count subsample stride


def _mm_chunks(a, b):
    # 512-aligned chunks: matmul output must not cross a PSUM bank boundary
    out = []
    s = a
    while s < b:
        e = min(s + 512, b)
        out.append((s, e))
        s = e
    assert all((y - x) % 2 == 0 and x % 512 == 0 for x, y in out), out
    return out


def _build_program(U):
    import concourse.bacc as bacc
    import concourse.mybir as mybir
    import concourse.tile as tile

    f32 = mybir.dt.float32
    mm_dt = getattr(mybir.dt, MM_DTYPE)
    W, HALF = _splits(U)
    NS = HALF // STRIDE            # count samples per tile

    nc = bacc.Bacc("TRN2", target_bir_lowering=False, debug=False, num_devices=NCORES)
    # host packs: row0..2 = x,y,z ; row3 = ones (v side) / -|c|^2/2 (c side)
    vc4 = nc.dram_tensor("vc4", [4, ROWS + W], mm_dt, kind="ExternalInput")
    out = nc.dram_tensor("idx", [NT // 8, 128, 8], f32, kind="ExternalOutput")

    NEG = -3.0e38
    mm = _mm_chunks(0, HALF)

    with tile.TileContext(nc) as tc:
        with (
            tc.tile_pool(name="const", bufs=1) as cpool,
            tc.tile_pool(name="sc", bufs=2) as scpool,
            tc.tile_pool(name="w", bufs=4) as wpool,
            tc.tile_pool(name="psum", bufs=1, space="PSUM") as ppool,
        ):
            vc_sb = cpool.tile([4, ROWS + W], mm_dt)
            nc.sync.dma_start(vc_sb[:], vc4[:])
            dummy = cpool.tile([128, 1], f32)
            nc.gpsimd.memset(dummy[:], 0.0)

            for t in range(NT):
                vT = vc_sb[:, t * 128:(t + 1) * 128]
                sc = scpool.tile([128, HALF], f32, tag="sc", name=f"sc{t}")
                sbB = scpool.tile([128, HALF], f32, tag="sb", name=f"sb{t}")

                # A-half PSUM in two pieces so the chained scans release
                # banks early (fine-grained PE/DVE pipelining in 8 banks)
                A1 = min(1024, HALF)
                psA1 = ppool.tile([128, A1], f32, tag="psA1", name=f"psA1_{t}")
                psA2 = (ppool.tile([128, HALF - A1], f32, tag="psA2",
                                   name=f"psA2_{t}") if HALF > A1 else None)
                psB = ppool.tile([128, HALF], f32, tag="psB", name=f"psB{t}")
                for (ca, cb) in mm:
                    dst = (psA1[:, ca:cb] if cb <= A1
                           else psA2[:, ca - A1:cb - A1])
                    nc.tensor.matmul(dst, vT,
                                     vc_sb[:, ROWS + ca:ROWS + cb],
                                     start=True, stop=True)
                for (ca, cb) in mm:
                    nc.tensor.matmul(psB[:, ca:cb], vT,
                                     vc_sb[:, ROWS + HALF + ca:ROWS + HALF + cb],
                                     start=True, stop=True)
                nc.scalar.copy(sbB[:], psB[:])

                # sc[t] = max(sc[t-1], psA[t], sbB[t]): running max of column
                # pairs {t, HALF+t}; r = sc[:, -1] is the global row max.
                nc.vector.tensor_tensor_scan(
                    sc[:, 0:A1], psA1[:], sbB[:, 0:A1], initial=NEG,
                    op0=mybir.AluOpType.max, op1=mybir.AluOpType.max)
                if psA2 is not None:
                    nc.vector.tensor_tensor_scan(
                        sc[:, A1:HALF], psA2[:], sbB[:, A1:HALF],
                        initial=sc[:, A1 - 1:A1],
                        op0=mybir.AluOpType.max, op1=mybir.AluOpType.max)

                # coarse count: #(subsampled sc < r) -> block index of first
                # pair-block containing r; host refines the 2*STRIDE columns.
                ko = wpool.tile([128, 8], f32, tag="ko", name=f"ko{t // 8}", bufs=2) \
                    if t % 8 == 0 else ko  # noqa: F821
                trash = wpool.tile([128, NS], f32, tag="tr", name=f"tr{t}", bufs=2)
                sub = sc[:, STRIDE - 1::STRIDE]
                nc.vector.scalar_tensor_tensor(
                    trash[:], sub, sc[:, HALF - 1:HALF],
                    dummy[:, 0:1].to_broadcast((128, NS)),
                    op0=mybir.AluOpType.is_lt, op1=mybir.AluOpType.bypass,
                    accum_out=ko[:, t % 8:t % 8 + 1])
                if t % 8 == 7:
                    nc.sync.dma_start(out[t // 8], ko[:])
    nc.compile()
    return nc


def _get_program(U):
    if U not in _PROGRAM_CACHE:
        _PROGRAM_CACHE[U] = _build_program(U)
    return _PROGRAM_CACHE[U]


def kernel(vertices, collider, collision_vertices, _want_trace=False):
    from concourse.bass_utils import run_bass_kernel_spmd

    v = np.ascontiguousarray(np.asarray(vertices), dtype=np.float32)     # [B,N,3]
    c = np.ascontiguousarray(np.asarray(collider), dtype=np.float32)     # [B,V,3]
    cvi = np.asarray(collision_vertices).astype(np.int64)                # [M]

    # dedup candidates, keeping first-occurrence order (exact tie semantics)
    u, first_pos = np.unique(cvi, return_index=True)
    order = np.argsort(first_pos)
    u = u[order]
    first_pos = first_pos[order].astype(np.int32)
    U = len(u)
    W, HALF = _splits(U)

    cv = c[:, u, :]                                               # [B,U,3]
    c2 = (cv * cv).sum(-1, dtype=np.float32)                      # [B,U]
    c2h = c2 * np.float32(-0.5)

    cv4_pad = np.zeros((B, 4, W), np.float32)
    cv4_pad[:, :3, :U] = cv.transpose(0, 2, 1)
    cv4_pad[:, 3, :U] = c2h
    cv4_pad[:, 3, U:] = np.float32(-5e29)   # poison padding scores

    in_maps = []
    for core in range(NCORES):
        b = core // 2
        r0 = (core % 2) * ROWS
        v4 = np.empty((4, ROWS), np.float32)
        v4[:3] = v[b, r0:r0 + ROWS, :].T
        v4[3] = 1.0
        in_maps.append({
            "vc4": np.ascontiguousarray(
                np.concatenate([v4, cv4_pad[b]], axis=1), dtype=np.float32),
        })

    nc = _get_program(U)
    res = run_bass_kernel_spmd(nc, in_maps, core_ids=list(range(NCORES)))

    # --- host refinement: device returns the coarse pair-block index; the
    # exact argmin among its 2*STRIDE candidate columns is recomputed here
    # with the reference's own fp32 arithmetic (also absorbs fp32r wobble).
    nwin = 2 * STRIDE
    nn = np.zeros((B, N), np.int32)
    for core in range(NCORES):
        b = core // 2
        r0 = (core % 2) * ROWS
        arr = res.results[core]["idx"].reshape(NT // 8, 128, 8)
        cblk = arr.transpose(0, 2, 1).reshape(-1)           # [tile, row] order
        cblk = np.clip(np.rint(cblk).astype(np.int64), 0, HALF // STRIDE - 1)
        t0 = cblk * STRIDE                                   # [ROWS]
        pairs = t0[:, None] + np.arange(STRIDE)              # [ROWS, STRIDE]
        slots = np.concatenate([pairs, pairs + HALF], 1)     # [ROWS, 2*STRIDE]
        valid = slots < U
        sl = np.minimum(slots, U - 1)
        vr = v[b, r0:r0 + ROWS, :]                           # [ROWS, 3]
        dot = np.einsum('rd,rkd->rk', vr, cv[b][sl], dtype=np.float32)
        d2 = c2[b][sl] - np.float32(2.0) * dot               # [ROWS, 2*STRIDE]
        d2 = np.where(valid, d2, np.float32(np.inf))
        win = np.argmin(d2, axis=1)                          # first min in slot order
        nn[b, r0:r0 + ROWS] = first_pos[sl[np.arange(ROWS), win]]
    batch_idx = np.broadcast_to(np.arange(B, dtype=np.int32)[:, None], nn.shape)
    outv = np.stack([batch_idx, nn], axis=-1).astype(np.int32)
    if _want_trace:
        return outv, (res, in_maps)
    return outv


# revision 19
# speedup vs baseline: 1.0742x; 1.0742x over previous
"""Exact KNN collision kernel for trn2 (8 NeuronCores).

nn[b,n] = argmin_m |vertices[b,n] - collider[b, cvi[m]]|^2, bit-exact fp32
scores with first-occurrence tie-breaking (matches the jax reference).

Per core (core c -> batch b=c//2, row-half h=c%2, 8192 rows, 64 tiles of 128):
  - PE: s = [v;1]^T @ [c; -|c|^2/2]  (K=4 fp32r matmul -> PSUM chunks)
    argmax_m s == argmin_m d2 exactly (d2 = -2s in fp32).
  - pass 1: running-max scan of s along candidates (tensor_tensor_scan max,
    split DVE/Pool, phase-chained via initial= carry) -> sc in SBUF.
  - r = max(r_dve_region, r_pool_region).
  - pass 2: counts of (sc < r) per chunk: ACT (Sign activation, scale=-1,
    bias=r, sum-accumulator) + DVE/Pool (scalar_tensor_tensor is_lt).
    Since sc is monotone per region, count == position of first occurrence;
    chunk counts compose: k = cnt1 + [cnt1==len1]*(cnt2 + [cnt2==len2]*(...)).
  - host maps dedup slot -> first position in collision_vertices.
"""
import os
import sys
import numpy as np

_BASS_PATH = "/opt/trn_rl_repo"
if _BASS_PATH not in sys.path:
    sys.path.insert(0, _BASS_PATH)

B, N, V, M = 4, 16384, 6890, 4096
NCORES = 8
ROWS = (B * N) // NCORES          # 8192 rows per core
NT = ROWS // 128                  # 64 row tiles

MM_DTYPE = os.environ.get("KNN_MM_DTYPE", "float32")

_PROGRAM_CACHE = {}


def _splits(U):
    """W = padded total columns (even halves); HALF per scan stream."""
    W = ((U + 3) // 4) * 4
    return W, W // 2


STRIDE = int(os.environ.get("KNN_STRIDE", "16"))   # count subsample stride


def _mm_chunks(a, b):
    # 512-aligned chunks: matmul output must not cross a PSUM bank boundary
    out = []
    s = a
    while s < b:
        e = min(s + 512, b)
        out.append((s, e))
        s = e
    assert all(x % 512 == 0 for x, y in out), out
    return out


def _build_program(U):
    import concourse.bacc as bacc
    import concourse.mybir as mybir
    import concourse.tile as tile

    f32 = mybir.dt.float32
    mm_dt = getattr(mybir.dt, MM_DTYPE)
    W, HALF = _splits(U)
    NS = HALF // STRIDE            # count samples per tile

    nc = bacc.Bacc("TRN2", target_bir_lowering=False, debug=False, num_devices=NCORES)
    # host packs: row0..2 = x,y,z ; row3 = ones (v side) / -|c|^2/2 (c side)
    vc4 = nc.dram_tensor("vc4", [4, ROWS + W], mm_dt, kind="ExternalInput")
    out = nc.dram_tensor("idx", [NT // 8, 128, 8], f32, kind="ExternalOutput")

    NEG = -3.0e38
    mm = _mm_chunks(0, HALF)

    with tile.TileContext(nc) as tc:
        with (
            tc.tile_pool(name="const", bufs=1) as cpool,
            tc.tile_pool(name="sc", bufs=2) as scpool,
            tc.tile_pool(name="w", bufs=4) as wpool,
            tc.tile_pool(name="psum", bufs=1, space="PSUM") as ppool,
        ):
            vc_sb = cpool.tile([4, ROWS + W], mm_dt)
            nc.sync.dma_start(vc_sb[:], vc4[:])
            dummy = cpool.tile([128, 1], f32)
            nc.gpsimd.memset(dummy[:], 0.0)

            for t in range(NT):
                vT = vc_sb[:, t * 128:(t + 1) * 128]
                sc = scpool.tile([128, HALF], f32, tag="sc", name=f"sc{t}")
                sbB = scpool.tile([128, HALF], f32, tag="sb", name=f"sb{t}")

                psA = ppool.tile([128, HALF], f32, tag="psA", name=f"psA{t}")
                psB = ppool.tile([128, HALF], f32, tag="psB", name=f"psB{t}")
                for (ca, cb) in mm:
                    nc.tensor.matmul(psA[:, ca:cb], vT,
                                     vc_sb[:, ROWS + ca:ROWS + cb],
                                     start=True, stop=True)
                for (ca, cb) in mm:
                    nc.tensor.matmul(psB[:, ca:cb], vT,
                                     vc_sb[:, ROWS + HALF + ca:ROWS + HALF + cb],
                                     start=True, stop=True)
                nc.scalar.copy(sbB[:], psB[:])

                # sc[t] = max(sc[t-1], psA[t], sbB[t]): running max of column
                # pairs {t, HALF+t}; r = sc[:, -1] is the global row max.
                nc.vector.tensor_tensor_scan(
                    sc[:], psA[:], sbB[:], initial=NEG,
                    op0=mybir.AluOpType.max, op1=mybir.AluOpType.max)

                # coarse count: #(subsampled sc < r) -> block index of first
                # pair-block containing r; host refines the 2*STRIDE columns.
                ko = wpool.tile([128, 8], f32, tag="ko", name=f"ko{t // 8}", bufs=2) \
                    if t % 8 == 0 else ko  # noqa: F821
                trash = wpool.tile([128, NS], f32, tag="tr", name=f"tr{t}", bufs=2)
                sub = sc[:, STRIDE - 1::STRIDE]
                nc.vector.scalar_tensor_tensor(
                    trash[:], sub, sc[:, HALF - 1:HALF],
                    dummy[:, 0:1].to_broadcast((128, NS)),
                    op0=mybir.AluOpType.is_lt, op1=mybir.AluOpType.bypass,
                    accum_out=ko[:, t % 8:t % 8 + 1])
                if t % 8 == 7:
                    nc.sync.dma_start(out[t // 8], ko[:])
    nc.compile()
    return nc


def _get_program(U):
    if U not in _PROGRAM_CACHE:
        _PROGRAM_CACHE[U] = _build_program(U)
    return _PROGRAM_CACHE[U]


def kernel(vertices, collider, collision_vertices, _want_trace=False):
    from concourse.bass_utils import run_bass_kernel_spmd

    v = np.ascontiguousarray(np.asarray(vertices), dtype=np.float32)     # [B,N,3]
    c = np.ascontiguousarray(np.asarray(collider), dtype=np.float32)     # [B,V,3]
    cvi = np.asarray(collision_vertices).astype(np.int64)                # [M]

    # dedup candidates, keeping first-occurrence order (exact tie semantics)
    u, first_pos = np.unique(cvi, return_index=True)
    order = np.argsort(first_pos)
    u = u[order]
    first_pos = first_pos[order].astype(np.int32)
    U = len(u)
    W, HALF = _splits(U)

    cv = c[:, u, :]                                               # [B,U,3]
    import jax.numpy as _jnp
    c2 = np.asarray(_jnp.sum(_jnp.asarray(cv) * _jnp.asarray(cv), axis=-1))
    c2h = c2 * np.float32(-0.5)

    cv4_pad = np.zeros((B, 4, W), np.float32)
    cv4_pad[:, :3, :U] = cv.transpose(0, 2, 1)
    cv4_pad[:, 3, :U] = c2h
    cv4_pad[:, 3, U:] = np.float32(-5e29)   # poison padding scores

    in_maps = []
    for core in range(NCORES):
        b = core // 2
        r0 = (core % 2) * ROWS
        v4 = np.empty((4, ROWS), np.float32)
        v4[:3] = v[b, r0:r0 + ROWS, :].T
        v4[3] = 1.0
        in_maps.append({
            "vc4": np.ascontiguousarray(
                np.concatenate([v4, cv4_pad[b]], axis=1), dtype=np.float32),
        })

    nc = _get_program(U)
    res = run_bass_kernel_spmd(nc, in_maps, core_ids=list(range(NCORES)))

    # --- host refinement: device returns the coarse pair-block index; the
    # exact argmin among its 2*STRIDE candidate columns is recomputed here
    # with the reference's own jax fp32 arithmetic (bit-matching tie cases).
    import jax
    import jax.numpy as jnp

    @jax.jit
    def _refine(vr, cvw, c2w, msk):
        d2 = c2w - 2.0 * jnp.einsum('rd,rkd->rk', vr, cvw)
        d2 = jnp.where(msk, d2, jnp.inf)
        return jnp.argmin(d2, axis=-1)

    nn = np.zeros((B, N), np.int32)
    for core in range(NCORES):
        b = core // 2
        r0 = (core % 2) * ROWS
        arr = res.results[core]["idx"].reshape(NT // 8, 128, 8)
        cblk = arr.transpose(0, 2, 1).reshape(-1)           # [tile, row] order
        cblk = np.clip(np.rint(cblk).astype(np.int64), 0, (HALF - 1) // STRIDE)
        pairs = cblk[:, None] * STRIDE + np.arange(STRIDE)   # [ROWS, STRIDE]
        slots = np.concatenate([pairs, pairs + HALF], 1)     # [ROWS, 2*STRIDE]
        valid = (np.concatenate([pairs, pairs], 1) < HALF) & (slots < U)
        sl = np.minimum(slots, U - 1)
        vr = v[b, r0:r0 + ROWS, :]                           # [ROWS, 3]
        win = np.asarray(_refine(jnp.asarray(vr), jnp.asarray(cv[b][sl]),
                                 jnp.asarray(c2[b][sl]), jnp.asarray(valid)))
        nn[b, r0:r0 + ROWS] = first_pos[sl[np.arange(ROWS), win]]
    batch_idx = np.broadcast_to(np.arange(B, dtype=np.int32)[:, None], nn.shape)
    outv = np.stack([batch_idx, nn], axis=-1).astype(np.int32)
    if _want_trace:
        return outv, (res, in_maps)
    return outv


# revision 20
# speedup vs baseline: 1.3899x; 1.2940x over previous
"""Exact KNN collision kernel for trn2 (8 NeuronCores).

nn[b,n] = argmin_m |vertices[b,n] - collider[b, cvi[m]]|^2, bit-exact fp32
scores with first-occurrence tie-breaking (matches the jax reference).

Per core (core c -> batch b=c//2, row-half h=c%2, 8192 rows, 64 tiles of 128):
  - PE: s = [v;1]^T @ [c; -|c|^2/2]  (K=4 fp32r matmul -> PSUM chunks)
    argmax_m s == argmin_m d2 exactly (d2 = -2s in fp32).
  - pass 1: running-max scan of s along candidates (tensor_tensor_scan max,
    split DVE/Pool, phase-chained via initial= carry) -> sc in SBUF.
  - r = max(r_dve_region, r_pool_region).
  - pass 2: counts of (sc < r) per chunk: ACT (Sign activation, scale=-1,
    bias=r, sum-accumulator) + DVE/Pool (scalar_tensor_tensor is_lt).
    Since sc is monotone per region, count == position of first occurrence;
    chunk counts compose: k = cnt1 + [cnt1==len1]*(cnt2 + [cnt2==len2]*(...)).
  - host maps dedup slot -> first position in collision_vertices.
"""
import os
import sys
import numpy as np

_BASS_PATH = "/opt/trn_rl_repo"
if _BASS_PATH not in sys.path:
    sys.path.insert(0, _BASS_PATH)

B, N, V, M = 4, 16384, 6890, 4096
NCORES = 8
ROWS = (B * N) // NCORES          # 8192 rows per core
NT = ROWS // 128                  # 64 row tiles

MM_DTYPE = os.environ.get("KNN_MM_DTYPE", "float32")

_PROGRAM_CACHE = {}


def _splits(U):
    """W = padded total columns (even halves); HALF per scan stream."""
    W = ((U + 3) // 4) * 4
    return W, W // 2


STRIDE = int(os.environ.get("KNN_STRIDE", "16"))   # count subsample stride


def _mm_chunks(a, b):
    # 512-aligned chunks: matmul output must not cross a PSUM bank boundary
    out = []
    s = a
    while s < b:
        e = min(s + 512, b)
        out.append((s, e))
        s = e
    assert all(x % 512 == 0 for x, y in out), out
    return out


def _build_program(U):
    import concourse.bacc as bacc
    import concourse.mybir as mybir
    import concourse.tile as tile

    f32 = mybir.dt.float32
    mm_dt = getattr(mybir.dt, MM_DTYPE)
    W, HALF = _splits(U)
    L = 1536 if W > 1536 else (W // 2) * 2          # left PSUM tile columns
    R = W - L
    NL, NR = L // 2, R // 2                          # pairs per side
    NS = HALF // STRIDE                              # count samples per tile

    nc = bacc.Bacc("TRN2", target_bir_lowering=False, debug=False, num_devices=NCORES)
    # host packs: row0..2 = x,y,z ; row3 = ones (v side) / -|c|^2/2 (c side)
    vc4 = nc.dram_tensor("vc4", [4, ROWS + W], mm_dt, kind="ExternalInput")
    out = nc.dram_tensor("idx", [NT // 8, 128, 8], f32, kind="ExternalOutput")

    NEG = -3.0e38
    mmL = _mm_chunks(0, L)
    mmR = _mm_chunks(0, R)

    with tile.TileContext(nc) as tc:
        with (
            tc.tile_pool(name="const", bufs=1) as cpool,
            tc.tile_pool(name="sc", bufs=2) as scpool,
            tc.tile_pool(name="w", bufs=4) as wpool,
            tc.tile_pool(name="psum", bufs=1, space="PSUM") as ppool,
        ):
            vc_sb = cpool.tile([4, ROWS + W], mm_dt)
            nc.sync.dma_start(vc_sb[:], vc4[:])
            dummy = cpool.tile([128, 1], f32)
            nc.gpsimd.memset(dummy[:], 0.0)

            for t in range(NT):
                vT = vc_sb[:, t * 128:(t + 1) * 128]
                sc = scpool.tile([128, HALF], f32, tag="sc", name=f"sc{t}")
                sbL = scpool.tile([128, NL], f32, tag="sbL", name=f"sbL{t}")
                sbR = scpool.tile([128, NR], f32, tag="sbR", name=f"sbR{t}")

                psL = ppool.tile([128, L], f32, tag="psL", name=f"psL{t}")
                psR = ppool.tile([128, R], f32, tag="psR", name=f"psR{t}")
                for (ca, cb) in mmL:
                    nc.tensor.matmul(psL[:, ca:cb], vT,
                                     vc_sb[:, ROWS + ca:ROWS + cb],
                                     start=True, stop=True)
                nc.scalar.copy(sbL[:], psL[:, 1::2])       # odd columns of left
                for (ca, cb) in mmR:
                    nc.tensor.matmul(psR[:, ca:cb], vT,
                                     vc_sb[:, ROWS + L + ca:ROWS + L + cb],
                                     start=True, stop=True)
                nc.scalar.copy(sbR[:], psR[:, 1::2])       # odd columns of right

                # adjacent-pair running max: sc[p] = max over slots [0, 2p+2)
                # (order-preserving: first pair achieving r == first slot // 2)
                nc.vector.tensor_tensor_scan(
                    sc[:, 0:NL], psL[:, 0::2], sbL[:], initial=NEG,
                    op0=mybir.AluOpType.max, op1=mybir.AluOpType.max)
                nc.vector.tensor_tensor_scan(
                    sc[:, NL:HALF], psR[:, 0::2], sbR[:],
                    initial=sc[:, NL - 1:NL],
                    op0=mybir.AluOpType.max, op1=mybir.AluOpType.max)

                # coarse count: #(subsampled sc < r) -> first pair-block with r;
                # host refines the 2*STRIDE consecutive candidate columns.
                ko = wpool.tile([128, 8], f32, tag="ko", name=f"ko{t // 8}", bufs=2) \
                    if t % 8 == 0 else ko  # noqa: F821
                trash = wpool.tile([128, NS], f32, tag="tr", name=f"tr{t}", bufs=2)
                nc.vector.scalar_tensor_tensor(
                    trash[:], sc[:, STRIDE - 1::STRIDE], sc[:, HALF - 1:HALF],
                    dummy[:, 0:1].to_broadcast((128, NS)),
                    op0=mybir.AluOpType.is_lt, op1=mybir.AluOpType.bypass,
                    accum_out=ko[:, t % 8:t % 8 + 1])
                if t % 8 == 7:
                    nc.sync.dma_start(out[t // 8], ko[:])
    nc.compile()
    return nc


def _get_program(U):
    if U not in _PROGRAM_CACHE:
        _PROGRAM_CACHE[U] = _build_program(U)
    return _PROGRAM_CACHE[U]


def kernel(vertices, collider, collision_vertices, _want_trace=False):
    from concourse.bass_utils import run_bass_kernel_spmd

    v = np.ascontiguousarray(np.asarray(vertices), dtype=np.float32)     # [B,N,3]
    c = np.ascontiguousarray(np.asarray(collider), dtype=np.float32)     # [B,V,3]
    cvi = np.asarray(collision_vertices).astype(np.int64)                # [M]

    # dedup candidates, keeping first-occurrence order (exact tie semantics)
    u, first_pos = np.unique(cvi, return_index=True)
    order = np.argsort(first_pos)
    u = u[order]
    first_pos = first_pos[order].astype(np.int32)
    U = len(u)
    W, HALF = _splits(U)

    cv = c[:, u, :]                                               # [B,U,3]
    import jax.numpy as _jnp
    c2 = np.asarray(_jnp.sum(_jnp.asarray(cv) * _jnp.asarray(cv), axis=-1))
    c2h = c2 * np.float32(-0.5)

    cv4_pad = np.zeros((B, 4, W), np.float32)
    cv4_pad[:, :3, :U] = cv.transpose(0, 2, 1)
    cv4_pad[:, 3, :U] = c2h
    cv4_pad[:, 3, U:] = np.float32(-5e29)   # poison padding scores

    in_maps = []
    for core in range(NCORES):
        b = core // 2
        r0 = (core % 2) * ROWS
        v4 = np.empty((4, ROWS), np.float32)
        v4[:3] = v[b, r0:r0 + ROWS, :].T
        v4[3] = 1.0
        in_maps.append({
            "vc4": np.ascontiguousarray(
                np.concatenate([v4, cv4_pad[b]], axis=1), dtype=np.float32),
        })

    nc = _get_program(U)
    res = run_bass_kernel_spmd(nc, in_maps, core_ids=list(range(NCORES)))

    # --- host refinement: device returns the coarse pair-block index; the
    # exact argmin among its 2*STRIDE candidate columns is recomputed here
    # with the reference's own jax fp32 arithmetic (bit-matching tie cases).
    import jax
    import jax.numpy as jnp

    @jax.jit
    def _refine(vr, cvw, c2w, msk):
        d2 = c2w - 2.0 * jnp.einsum('rd,rkd->rk', vr, cvw)
        d2 = jnp.where(msk, d2, jnp.inf)
        return jnp.argmin(d2, axis=-1)

    nn = np.zeros((B, N), np.int32)
    for core in range(NCORES):
        b = core // 2
        r0 = (core % 2) * ROWS
        arr = res.results[core]["idx"].reshape(NT // 8, 128, 8)
        cblk = arr.transpose(0, 2, 1).reshape(-1)           # [tile, row] order
        cblk = np.clip(np.rint(cblk).astype(np.int64), 0, (HALF - 1) // STRIDE)
        slots = cblk[:, None] * (2 * STRIDE) + np.arange(2 * STRIDE)
        valid = slots < U
        sl = np.minimum(slots, U - 1)
        vr = v[b, r0:r0 + ROWS, :]                           # [ROWS, 3]
        win = np.asarray(_refine(jnp.asarray(vr), jnp.asarray(cv[b][sl]),
                                 jnp.asarray(c2[b][sl]), jnp.asarray(valid)))
        nn[b, r0:r0 + ROWS] = first_pos[sl[np.arange(ROWS), win]]
    batch_idx = np.broadcast_to(np.arange(B, dtype=np.int32)[:, None], nn.shape)
    outv = np.stack([batch_idx, nn], axis=-1).astype(np.int32)
    if _want_trace:
        return outv, (res, in_maps)
    return outv
